# revision 1
# baseline (speedup 1.0000x reference)
"""Multi-head attention (B=2, S=2048, E=2048, H=16, causal) on 8 TRN2 NeuronCores.

Sharding: 8 cores = 2 batch shards x 4 head-group shards (4 heads / 512
features each).  Each core runs the full attention stack for its (batch,
head-group) and produces a partial [S, E] output through its row-block of
Wo; the host sums the 4 partials per batch.

All matmuls run as float32r (full PE rate for free dim >= 256).
"""

import numpy as np

import concourse.bacc as bacc
import concourse.mybir as mybir
import concourse.tile as tile
from concourse import bass_utils

B, S, E, H = 2, 2048, 2048, 16
D = 128                    # head dim
HL = 4                     # heads per core
F = HL * D                 # local features = 512
EO = E // 128              # 16 contraction chunks
EG = 2                     # eo chunks per DMA group
TT = 256                   # phase-1 token tile
IT = 512                   # phase-2 query tile
F32 = mybir.dt.float32
F32R = mybir.dt.float32r
EXP = mybir.ActivationFunctionType.Exp
SCALE = 1.0 / float(np.sqrt(D))

_CACHE = {}


def _build():
    nc = bacc.Bacc("TRN2", target_bir_lowering=False, debug=False)
    xT = nc.dram_tensor("xT", [E, S], F32, kind="ExternalInput").ap()
    wqT = nc.dram_tensor("wqT", [E, F], F32, kind="ExternalInput").ap()
    wkT = nc.dram_tensor("wkT", [E, F], F32, kind="ExternalInput").ap()
    wvT = nc.dram_tensor("wvT", [E, F], F32, kind="ExternalInput").ap()
    woT = nc.dram_tensor("woT", [F, E], F32, kind="ExternalInput").ap()
    # causal mask pairs: [pair, 128, 2, IT]
    cmask = nc.dram_tensor("cmask", [2, 128, 2, IT], F32, kind="ExternalInput").ap()
    y = nc.dram_tensor("y", [S, E], F32, kind="ExternalOutput").ap()

    xT_t = xT.rearrange("(eo ei) t -> ei eo t", ei=128).bitcast(F32R)
    wqT_t = wqT.rearrange("(eo ei) f -> ei eo f", ei=128).bitcast(F32R)
    wkT_t = wkT.rearrange("(eo ei) f -> ei eo f", ei=128).bitcast(F32R)
    wvT_t = wvT.rearrange("(eo ei) f -> ei eo f", ei=128).bitcast(F32R)
    woT_t = woT.rearrange("(fc fi) e -> fi fc e", fi=128).bitcast(F32R)

    with tile.TileContext(nc) as tc:
        with tc.tile_pool(name="persist", bufs=1) as persist:
            qT = persist.tile([128, HL, S], F32R, tag="qT")
            kT = persist.tile([128, HL, S], F32R, tag="kT")
            vN = persist.tile([128, S // 128, F], F32R, tag="vN")
            maskT = persist.tile([128, 2, 2, IT], F32, tag="maskT")
            onesT_f = persist.tile([128, 1], F32, tag="onesT_f")
            onesT = persist.tile([128, 1], F32R, tag="onesT")

            nc.vector.memset(onesT_f[:], 1.0)
            nc.vector.tensor_copy(onesT[:], onesT_f[:])

            # ---------- phase 1: q/k/v projections (two f-half passes) ----
            with (
                tc.tile_pool(name="wres_q", bufs=2) as wpool_q,
                tc.tile_pool(name="wres_kv", bufs=1) as wpool_kv,
                tc.tile_pool(name="xstream", bufs=2) as xpool,
                tc.tile_pool(name="ps_qk", bufs=5, space="PSUM") as ps_qk,
                tc.tile_pool(name="ps_v", bufs=3, space="PSUM") as ps_v,
            ):
                for fp in range(2):
                    f0 = fp * 256
                    wq_res = wpool_q.tile([128, EO, 256], F32R, tag="wq")
                    wk_res = wpool_kv.tile([128, EO, 256], F32R, tag="wk")
                    wv_res = wpool_kv.tile([128, EO, 256], F32R, tag="wv")
                    xt0 = xpool.tile([128, EO, TT], F32R, tag="xt")
                    # issue in consumption order: wq/x first, then wk, wv
                    for g0 in range(0, EO, EG):
                        nc.sync.dma_start(
                            wq_res[:, g0:g0 + EG, :],
                            wqT_t[:, g0:g0 + EG, f0:f0 + 256],
                        )
                        nc.sync.dma_start(
                            xt0[:, g0:g0 + EG, :], xT_t[:, g0:g0 + EG, 0:TT]
                        )
                    for g0 in range(0, EO, EG):
                        nc.sync.dma_start(
                            wk_res[:, g0:g0 + EG, :],
                            wkT_t[:, g0:g0 + EG, f0:f0 + 256],
                        )
                    for g0 in range(0, EO, EG):
                        nc.sync.dma_start(
                            wv_res[:, g0:g0 + EG, :],
                            wvT_t[:, g0:g0 + EG, f0:f0 + 256],
                        )
                    for tt in range(S // TT):
                        t0 = tt * TT
                        if fp == 0 and tt == 4:
                            nc.sync.dma_start(
                                maskT[:], cmask.rearrange("q p m i -> p q m i")
                            )
                        if tt == 0:
                            xt = xt0
                        else:
                            xt = xpool.tile([128, EO, TT], F32R, tag="xt")
                            for g0 in range(0, EO, EG):
                                nc.sync.dma_start(
                                    xt[:, g0:g0 + EG, :],
                                    xT_t[:, g0:g0 + EG, t0:t0 + TT],
                                )
                        for wres, dst in ((wq_res, qT), (wk_res, kT)):
                            for fc in range(2):
                                ps = ps_qk.tile([128, TT], F32, tag="pqk")
                                for eo in range(EO):
                                    nc.tensor.matmul(
                                        ps[:],
                                        wres[:, eo, fc * 128:(fc + 1) * 128],
                                        xt[:, eo, :],
                                        start=(eo == 0),
                                        stop=(eo == EO - 1),
                                    )
                                nc.vector.tensor_copy(
                                    dst[:, fp * 2 + fc, t0:t0 + TT], ps[:]
                                )
                        for tc2 in range(TT // 128):
                            ps = ps_v.tile([128, 256], F32, tag="pv")
                            for eo in range(EO):
                                nc.tensor.matmul(
                                    ps[:],
                                    xt[:, eo, tc2 * 128:(tc2 + 1) * 128],
                                    wv_res[:, eo, :],
                                    start=(eo == 0),
                                    stop=(eo == EO - 1),
                                )
                            nc.vector.tensor_copy(
                                vN[:, (t0 // 128) + tc2, f0:f0 + 256], ps[:]
                            )

            # ---------- phase 2: attention per head ----------------------
            with tc.tile_pool(name="wo", bufs=1) as wo_pool:
                wo_res = wo_pool.tile([128, HL, E], F32R, tag="wo")
                outT = wo_pool.tile([128, HL, S], F32R, tag="outT")
                for g0 in range(0, HL, 2):
                    nc.sync.dma_start(
                        wo_res[:, g0:g0 + 2, :], woT_t[:, g0:g0 + 2, :]
                    )

                with (
                    tc.tile_pool(name="ph2", bufs=6) as epool,
                    tc.tile_pool(name="ph2t", bufs=4) as tpool,
                    tc.tile_pool(name="ph2b", bufs=2) as small,
                    tc.tile_pool(name="ps_s", bufs=4, space="PSUM") as ps_s,
                    tc.tile_pool(name="ps_o", bufs=2, space="PSUM") as ps_o,
                    tc.tile_pool(name="ps_r", bufs=1, space="PSUM") as ps_r,
                    tc.tile_pool(name="ps_yb", bufs=1, space="PSUM") as ps_yb,
                    tc.tile_pool(name="ystb", bufs=2) as ystb_pool,
                ):
                    ready_y = []
                    done_y = set()

                    def emit_y_group():
                        tcb, et = ready_y.pop(0)
                        done_y.add((tcb, et))
                        Yb = ps_yb.tile([128, 512], F32, tag="Yb")
                        for fc in range(HL):
                            nc.tensor.matmul(
                                Yb[:],
                                outT[:, fc, tcb * 128:(tcb + 1) * 128],
                                wo_res[:, fc, et * 512:(et + 1) * 512],
                                start=(fc == 0),
                                stop=(fc == HL - 1),
                            )
                        yb = ystb_pool.tile([128, 512], F32, tag="yb")
                        nc.vector.tensor_copy(yb[:], Yb[:])
                        nc.sync.dma_start(
                            y[tcb * 128:(tcb + 1) * 128,
                              et * 512:(et + 1) * 512],
                            yb[:],
                        )
                    for p in range(S // IT):
                        i0 = p * IT
                        for h in range(HL):
                            h0 = h * 128
                            njc = (i0 + IT) // 128
                            O = ps_o.tile([128, IT], F32, tag="O")
                            R = ps_r.tile([1, IT], F32, tag="R")

                            def emit_scores(jc):
                                q_off = jc - (i0 // 128)
                                # diag chunk q: columns i < 128*q are fully
                                # masked -- compute only the valid slice
                                # (clamped so the free dim stays >= 256 for
                                # the f32r fast path)
                                off = 0 if q_off < 0 else min(128 * q_off, 256)
                                Sps = ps_s.tile([128, IT], F32, tag="S")
                                nc.tensor.matmul(
                                    Sps[:, off:],
                                    kT[:, h, jc * 128:(jc + 1) * 128],
                                    qT[:, h, i0 + off:i0 + IT],
                                    start=True,
                                    stop=True,
                                )
                                Et = epool.tile([128, IT], F32R, tag="E")
                                if q_off < 0:
                                    nc.scalar.activation(
                                        Et[:], Sps[:], EXP, scale=SCALE
                                    )
                                else:
                                    Etmp = tpool.tile([128, IT], F32, tag="Etmp")
                                    nc.scalar.activation(
                                        Etmp[:, off:], Sps[:, off:], EXP,
                                        scale=SCALE,
                                    )
                                    nc.vector.tensor_mul(
                                        Et[:, off:], Etmp[:, off:],
                                        maskT[:, q_off // 2, q_off % 2, off:],
                                    )
                                return Et, off

                            def emit_av(jc, Et, off):
                                nc.tensor.matmul(
                                    O[:, off:],
                                    vN[:, jc, h0:h0 + 128],
                                    Et[:, off:],
                                    start=(jc == 0),
                                    stop=(jc == njc - 1),
                                )
                                nc.tensor.matmul(
                                    R[:, off:],
                                    onesT[:],
                                    Et[:, off:],
                                    start=(jc == 0),
                                    stop=(jc == njc - 1),
                                )

                            # scores/exp run 4 chunks ahead of attn@v/rowsum
                            pending = []
                            for jc in range(njc):
                                Et, off = emit_scores(jc)
                                pending.append((jc, Et, off))
                                if jc == 2 and ready_y:
                                    emit_y_group()
                                if len(pending) > 3:
                                    emit_av(*pending.pop(0))
                            for item in pending:
                                emit_av(*item)
                            rec = small.tile([1, IT], F32, tag="rec")
                            nc.vector.reciprocal(rec[:], R[:])
                            RB = small.tile([128, IT], F32, tag="RB")
                            nc.gpsimd.partition_broadcast(RB[:], rec[:])
                            if h == HL - 1 and p == S // IT - 1:
                                # free the last O/R banks early so phase-3's
                                # psum pool isn't gated on the recip chain
                                Ocp = small.tile([128, IT], F32, tag="Ocp")
                                nc.vector.tensor_copy(Ocp[:], O[:])
                                nc.vector.tensor_mul(
                                    outT[:, h, i0:i0 + IT], Ocp[:], RB[:]
                                )
                            else:
                                nc.vector.tensor_mul(
                                    outT[:, h, i0:i0 + IT], O[:], RB[:]
                                )
                            if h == HL - 1:
                                for tcb_r in range(4 * p, 4 * p + 4):
                                    for et_r in range(E // 512):
                                        ready_y.append((tcb_r, et_r))

                # ------ phase 3: output projection ------------------------
                with (
                    tc.tile_pool(name="yst3", bufs=6) as yst_pool,
                    tc.tile_pool(name="ps_y", bufs=8, space="PSUM") as ps_y,
                ):
                    for tcb in range(S // 128):
                        tb0 = tcb * 128
                        for et in range(E // 512):
                            if (tcb, et) in done_y:
                                continue
                            Y = ps_y.tile([128, 512], F32, tag="Y")
                            for fc in range(HL):
                                nc.tensor.matmul(
                                    Y[:],
                                    outT[:, fc, tb0:tb0 + 128],
                                    wo_res[:, fc, et * 512:(et + 1) * 512],
                                    start=(fc == 0),
                                    stop=(fc == HL - 1),
                                )
                            yst = yst_pool.tile([128, 512], F32, tag="yst")
                            if et % 2 == 1:
                                nc.vector.tensor_copy(yst[:], Y[:])
                            else:
                                nc.scalar.copy(yst[:], Y[:])
                            nc.sync.dma_start(
                                y[tb0:tb0 + 128,
                                  et * 512:(et + 1) * 512],
                                yst[:],
                            )
    nc.compile()
    return nc


def _get_nc():
    if "nc" not in _CACHE:
        _CACHE["nc"] = _build()
    return _CACHE["nc"]


def make_in_maps(x, Wq, Wk, Wv, Wo):
    x = np.asarray(x, np.float32)
    Wq = np.asarray(Wq, np.float32)
    Wk = np.asarray(Wk, np.float32)
    Wv = np.asarray(Wv, np.float32)
    Wo = np.asarray(Wo, np.float32)

    jj = np.arange(128, dtype=np.int64)[:, None]
    ii = np.arange(IT, dtype=np.int64)[None, :]
    cm = np.stack(
        [(128 * q + jj <= ii).astype(np.float32) for q in range(4)]
    )  # [4, 128, IT]
    cmask = np.ascontiguousarray(
        cm.reshape(2, 2, 128, IT).transpose(0, 2, 1, 3)
    )  # [pair, 128, 2, IT]

    xTs = [np.ascontiguousarray(x[b].T) for b in range(B)]
    in_maps = []
    for c in range(8):
        b, g = c // 4, c % 4
        fsl = slice(F * g, F * (g + 1))
        in_maps.append({
            "xT": xTs[b],
            "wqT": np.ascontiguousarray(Wq[fsl, :].T),
            "wkT": np.ascontiguousarray(Wk[fsl, :].T),
            "wvT": np.ascontiguousarray(Wv[fsl, :].T),
            "woT": np.ascontiguousarray(Wo[:, fsl].T),
            "cmask": cmask,
        })
    return in_maps


def combine_outputs(results):
    out = np.empty((B, S, E), np.float32)
    for b in range(B):
        acc = results[4 * b]["y"].astype(np.float32).copy()
        for g in range(1, 4):
            acc += results[4 * b + g]["y"]
        out[b] = acc
    return out


def kernel(x, Wq, Wk, Wv, Wo):
    import time as _time

    nc = _get_nc()
    in_maps = make_in_maps(x, Wq, Wk, Wv, Wo)
    last_exc = None
    for attempt in range(3):
        if attempt:
            # transient device wedge (e.g. NRT_EXEC_UNIT_UNRECOVERABLE) --
            # wait for recovery before retrying
            _time.sleep(30 * attempt)
        try:
            res = bass_utils.run_bass_kernel_spmd(
                nc, in_maps, core_ids=list(range(8))
            )
            return combine_outputs(res.results)
        except Exception as exc:
            last_exc = exc
    raise last_exc



# revision 25
# speedup vs baseline: 1.1647x; 1.1647x over previous
"""Multi-head attention (B=2, S=2048, E=2048, H=16, causal) on 8 TRN2 NeuronCores.

Sharding: 8 cores = 2 batch shards x 4 head-group shards (4 heads / 512
features each).  Each core runs the full attention stack for its (batch,
head-group) and produces a partial [S, E] output through its row-block of
Wo; the host sums the 4 partials per batch.

Schedule: one fused software pipeline.  x streams once through SBUF in four
512-token tiles; attention for query-tile p is woven between the projection
PSUM-groups of token-tile p+1, so softmax exp/mask/row-sum latency hides
under projection matmuls.  Output-projection groups backfill the tensor
engine during the final attention stretch.

All matmul data is bf16 (f32 PSUM accumulation).  Softmax row-sums are
accumulated chunk-wise on the vector engine and reduced across partitions
with one gpsimd partition_all_reduce per (query-tile, head) - no
tensor-engine row-sum matmuls.
"""

import numpy as np

import concourse.bacc as bacc
import concourse.bass_isa as bass_isa
import concourse.mybir as mybir
import concourse.tile as tile
from concourse import bass_utils

B, S, E, H = 2, 2048, 2048, 16
D = 128                    # head dim
HL = 4                     # heads per core
F = HL * D                 # local features = 512
EO = E // 128              # 16 contraction chunks
TT = 512                   # token tile (phase 1) == query tile (phase 2)
NT = S // TT               # 4 tiles
F32 = mybir.dt.float32
BF16 = mybir.dt.bfloat16
EXP = mybir.ActivationFunctionType.Exp
SCALE = 1.0 / float(np.sqrt(D))

_CACHE = {}


def _build():
    nc = bacc.Bacc("TRN2", target_bir_lowering=False, debug=False)
    xT = nc.dram_tensor("xT", [E, S], BF16, kind="ExternalInput").ap()
    wqT = nc.dram_tensor("wqT", [E, F], BF16, kind="ExternalInput").ap()
    wkT = nc.dram_tensor("wkT", [E, F], BF16, kind="ExternalInput").ap()
    wvT = nc.dram_tensor("wvT", [E, F], BF16, kind="ExternalInput").ap()
    woT = nc.dram_tensor("woT", [F, E], BF16, kind="ExternalInput").ap()
    tri = nc.dram_tensor("tri", [128, 128], BF16, kind="ExternalInput").ap()
    y = nc.dram_tensor("y", [S, E], BF16, kind="ExternalOutput").ap()

    xT_t = xT.rearrange("(eo ei) t -> ei eo t", ei=128)
    wqT_t = wqT.rearrange("(eo ei) f -> ei eo f", ei=128)
    wkT_t = wkT.rearrange("(eo ei) f -> ei eo f", ei=128)
    wvT_t = wvT.rearrange("(eo ei) f -> ei eo f", ei=128)
    woT_t = woT.rearrange("(fc fi) e -> fi fc e", fi=128)

    with tile.TileContext(nc) as tc:
        with (
            tc.tile_pool(name="persist", bufs=1) as persist,
            tc.tile_pool(name="et", bufs=10) as epool,
            tc.tile_pool(name="esum", bufs=3) as spool,
            tc.tile_pool(name="rall", bufs=2) as rpool,
            tc.tile_pool(name="ystg", bufs=4) as ypool,
            tc.tile_pool(name="ps_s", bufs=3, space="PSUM") as ps_s,
            tc.tile_pool(name="ps_o", bufs=3, space="PSUM") as ps_o,
        ):
            qT = persist.tile([128, HL, S], BF16, tag="qT", name="qT")
            kT = persist.tile([128, HL, S], BF16, tag="kT", name="kT")
            vN = persist.tile([128, S // 128, F], BF16, tag="vN", name="vN")
            outT = persist.tile([128, HL, S], BF16, tag="outT", name="outT")
            wq_r = persist.tile([128, EO, F], BF16, tag="wq_r", name="wq_r")
            wk_r = persist.tile([128, EO, F], BF16, tag="wk_r", name="wk_r")
            wv_r = persist.tile([128, EO, F], BF16, tag="wv_r", name="wv_r")
            wo_r = persist.tile([128, HL, E], BF16, tag="wo_r", name="wo_r")
            triT = persist.tile([128, 128], BF16, tag="triT", name="triT")

            # ---- input DMA: wq/x-tile0 interleaved for fast start ----------
            pp = tc.alloc_tile_pool(name="pp", bufs=2, space="PSUM")
            xpool = tc.alloc_tile_pool(name="xstream", bufs=2)
            xt0 = xpool.tile([128, EO, TT], BF16, tag="xt", name="xt0")
            first_pieces = [(0, 1), (1, 2), (2, 4), (4, 8), (8, 12), (12, 16)]
            for lo, hi in first_pieces:
                nc.sync.dma_start(wq_r[:, lo:hi, :], wqT_t[:, lo:hi, :])
                nc.sync.dma_start(xt0[:, lo:hi, :], xT_t[:, lo:hi, 0:TT])
            for g0 in range(0, EO, 4):
                nc.sync.dma_start(wk_r[:, g0:g0 + 4, :], wkT_t[:, g0:g0 + 4, :])
            for g0 in range(0, EO, 4):
                nc.sync.dma_start(wv_r[:, g0:g0 + 4, :], wvT_t[:, g0:g0 + 4, :])
            nc.sync.dma_start(triT[:], tri[:])

            # ---------------- attention pipeline machinery ------------------
            jobs = [
                (p, h, jc)
                for p in range(NT)
                for h in range(HL)
                for jc in range(4 * (p + 1))
            ]
            combo = {}       # (p, h) -> {"O": psum tile, "esum": sbuf tile}
            pending = []     # scores emitted, attn@v not yet (LAG deep)
            ready_y = []
            done_y = set()
            ycount = [0]
            since_y = [0]
            ps_y_cell = [None]
            LAG = 5

            def emit_y_group(alt_pool=None):
                tcb, et = ready_y.pop(0)
                done_y.add((tcb, et))
                pool_ = alt_pool if alt_pool is not None else ps_y_cell[0]
                tag_ = "O" if alt_pool is not None else "Yb"
                Yb = pool_.tile([128, 512], F32, tag=tag_, name="Yb")
                for fc in range(HL):
                    nc.tensor.matmul(
                        Yb[:],
                        outT[:, fc, tcb * 128:(tcb + 1) * 128],
                        wo_r[:, fc, et * 512:(et + 1) * 512],
                        start=(fc == 0),
                        stop=(fc == HL - 1),
                    )
                yb = ypool.tile([128, 512], BF16, tag="yb", name="yb")
                if ycount[0] % 2 == 0:
                    nc.scalar.copy(yb[:], Yb[:])
                else:
                    nc.vector.tensor_copy(yb[:], Yb[:])
                ycount[0] += 1
                nc.sync.dma_start(
                    y[tcb * 128:(tcb + 1) * 128, et * 512:(et + 1) * 512],
                    yb[:],
                )

            def emit_scores(p, h, jc):
                q_off = jc - 4 * p
                off = 0 if q_off <= 0 else 128 * q_off
                st = combo.setdefault(
                    (p, h),
                    {
                        "O": ps_o.tile([128, TT], F32, tag="O", name="O"),
                        "esum": spool.tile(
                            [128, TT], BF16, tag="esum", name="esum"
                        ),
                    },
                )
                Sps = ps_s.tile([128, TT], F32, tag="S", name="S")
                nc.tensor.matmul(
                    Sps[:, off:],
                    kT[:, h, jc * 128:(jc + 1) * 128],
                    qT[:, h, p * TT + off:p * TT + TT],
                    start=True,
                    stop=True,
                )
                Et = epool.tile([128, TT], BF16, tag="E", name="E")
                nc.scalar.activation(
                    Et[:, off:], Sps[:, off:], EXP, scale=SCALE
                )
                if q_off >= 0:
                    # triangle mask on the 128-wide diagonal block
                    nc.gpsimd.tensor_mul(
                        Et[:, off:off + 128], Et[:, off:off + 128], triT[:]
                    )
                # softmax row-sum accumulation (chunk-wise, on DVE)
                esum = st["esum"]
                if jc == 0:
                    nc.vector.tensor_copy(esum[:], Et[:])
                else:
                    nc.vector.tensor_add(
                        esum[:, off:], esum[:, off:], Et[:, off:]
                    )
                return Et, off

            def emit_av(p, h, jc, Et, off):
                njc = 4 * (p + 1)
                st = combo[(p, h)]
                nc.tensor.matmul(
                    st["O"][:, off:],
                    vN[:, jc, h * 128:(h + 1) * 128],
                    Et[:, off:],
                    start=(jc == 0),
                    stop=(jc == njc - 1),
                )
                if jc == njc - 1:
                    finish_combo(p, h)

            def finish_combo(p, h):
                st = combo.pop((p, h))
                Rall = rpool.tile([128, TT], F32, tag="Rall", name="Rall")
                nc.gpsimd.partition_all_reduce(
                    Rall[:], st["esum"][:], 128, bass_isa.ReduceOp.add
                )
                RB = rpool.tile([128, TT], F32, tag="RB", name="RB")
                nc.vector.reciprocal(RB[:], Rall[:])
                nc.vector.tensor_mul(
                    outT[:, h, p * TT:p * TT + TT], st["O"][:], RB[:]
                )
                if h == HL - 1:
                    for tcb_r in range(4 * p, 4 * p + 4):
                        for et_r in range(E // 512):
                            ready_y.append((tcb_r, et_r))
                    since_y[0] = -4   # let the norm chain land first

            def emit_job(job):
                pending.append((*job, *emit_scores(*job)))
                if len(pending) >= LAG:
                    emit_av(*pending.pop(0))

            def maybe_y():
                # near the end, keep 4 groups in reserve to pad the PE while
                # the final combo's normalization chain completes
                since_y[0] += 1
                reserve = 4 if len(jobs) - job_pos[0] < 24 else 0
                if (ps_y_cell[0] is not None and len(ready_y) > reserve
                        and since_y[0] >= 1):
                    emit_y_group()
                    since_y[0] = 0

            job_pos = [0]

            def weave(quota):
                while quota > 0 and job_pos[0] < len(jobs):
                    emit_job(jobs[job_pos[0]])
                    job_pos[0] += 1
                    quota -= 1

            # ---------------- fused projection + attention loop -------------
            copy_flip = [0]

            def drain_copy(dst_ap, ps_ap):
                # alternate PSUM->SBUF bf16 copies between ACT and DVE
                if copy_flip[0] % 2 == 0:
                    nc.scalar.copy(dst_ap, ps_ap)
                else:
                    nc.vector.tensor_copy(dst_ap, ps_ap)
                copy_flip[0] += 1

            for tt in range(NT):
                t0 = tt * TT
                eligible = sum(16 * (p + 1) for p in range(tt))
                if tt == 0:
                    xt = xt0
                else:
                    xt = xpool.tile([128, EO, TT], BF16, tag="xt",
                                    name=f"xt{tt}")
                    for g0 in range(0, EO, 4):
                        nc.sync.dma_start(
                            xt[:, g0:g0 + 4, :],
                            xT_t[:, g0:g0 + 4, t0:t0 + TT],
                        )
                if tt == 1:
                    # wo needed from first y emission (after phase 1)
                    for g0 in range(0, HL, 2):
                        nc.sync.dma_start(
                            wo_r[:, g0:g0 + 2, :], woT_t[:, g0:g0 + 2, :]
                        )
                group = 0
                if tt == 0:
                    # tile 0 is DMA-paced: interleave all four q-groups (and
                    # k-group pairs) per eo-piece so the PE consumes wq/x0/wk
                    # at DMA-arrival rate.  fc2/fc3 banks borrow the idle
                    # attention O ring.
                    pieces = [(0, 1), (1, 2), (2, 4), (4, 8), (8, 12),
                              (12, 16)]
                    banks = [
                        pp.tile([128, TT], F32, tag="pp", name="pqk"),
                        pp.tile([128, TT], F32, tag="pp", name="pqk"),
                        ps_o.tile([128, TT], F32, tag="O", name="pqk"),
                        ps_o.tile([128, TT], F32, tag="O", name="pqk"),
                    ]
                    for lo, hi in pieces:
                        for fc in range(4):
                            for eo in range(lo, hi):
                                nc.tensor.matmul(
                                    banks[fc][:],
                                    wq_r[:, eo, fc * 128:(fc + 1) * 128],
                                    xt[:, eo, :],
                                    start=(eo == 0),
                                    stop=(eo == EO - 1),
                                )
                    for fc in range(4):
                        drain_copy(qT[:, fc, t0:t0 + TT], banks[fc][:])
                    for fcp in range(2):
                        kb = [
                            pp.tile([128, TT], F32, tag="pp", name="pqk"),
                            pp.tile([128, TT], F32, tag="pp", name="pqk"),
                        ]
                        for lo, hi in pieces:
                            for i, fc in enumerate((2 * fcp, 2 * fcp + 1)):
                                for eo in range(lo, hi):
                                    nc.tensor.matmul(
                                        kb[i][:],
                                        wk_r[:, eo, fc * 128:(fc + 1) * 128],
                                        xt[:, eo, :],
                                        start=(eo == 0),
                                        stop=(eo == EO - 1),
                                    )
                        for i, fc in enumerate((2 * fcp, 2 * fcp + 1)):
                            drain_copy(kT[:, fc, t0:t0 + TT], kb[i][:])
                    group = 8
                else:
                    for wres, dst in ((wq_r, qT), (wk_r, kT)):
                        for fc in range(HL):
                            ps = pp.tile([128, TT], F32, tag="pp", name="pqk")
                            for eo in range(EO):
                                nc.tensor.matmul(
                                    ps[:],
                                    wres[:, eo, fc * 128:(fc + 1) * 128],
                                    xt[:, eo, :],
                                    start=(eo == 0),
                                    stop=(eo == EO - 1),
                                )
                            drain_copy(dst[:, fc, t0:t0 + TT], ps[:])
                            group += 1
                            weave(
                                -(-(eligible - job_pos[0]) // (12 - group + 1))
                            )
                for tc2 in range(TT // 128):
                    ps = pp.tile([128, F], F32, tag="pp", name="pv")
                    for eo in range(EO):
                        nc.tensor.matmul(
                            ps[:],
                            xt[:, eo, tc2 * 128:(tc2 + 1) * 128],
                            wv_r[:, eo, :],
                            start=(eo == 0),
                            stop=(eo == EO - 1),
                        )
                    drain_copy(vN[:, (t0 // 128) + tc2, :], ps[:])
                    group += 1
                    if group < 12:
                        weave(-(-(eligible - job_pos[0]) // (12 - group + 1)))

            xpool.release()
            pp.release()
            ps_y_cell[0] = tc.alloc_tile_pool(name="ps_y", bufs=2,
                                              space="PSUM")

            # ------- remaining attention jobs + output projection -----------
            while job_pos[0] < len(jobs):
                emit_job(jobs[job_pos[0]])
                job_pos[0] += 1
                maybe_y()
            while pending:
                emit_av(*pending.pop(0))
                maybe_y()
            ydrain = 0
            while ready_y:
                # rotate across ps_y (2 banks) and the now-idle ps_o ring
                emit_y_group(ps_o if ydrain % 2 else None)
                ydrain += 1
            ps_y_cell[0].release()
    nc.compile()
    return nc


def _get_nc():
    if "nc" not in _CACHE:
        _CACHE["nc"] = _build()
    return _CACHE["nc"]


def make_in_maps(x, Wq, Wk, Wv, Wo):
    import ml_dtypes

    bf16 = ml_dtypes.bfloat16
    x = np.asarray(x, np.float32)
    Wq = np.asarray(Wq, np.float32)
    Wk = np.asarray(Wk, np.float32)
    Wv = np.asarray(Wv, np.float32)
    Wo = np.asarray(Wo, np.float32)

    jj = np.arange(128)[:, None]
    ii = np.arange(128)[None, :]
    tri = (jj <= ii).astype(bf16)  # key j valid for query i iff j <= i

    xTs = [np.ascontiguousarray(x[b].T).astype(bf16) for b in range(B)]
    in_maps = []
    for c in range(8):
        b, g = c // 4, c % 4
        fsl = slice(F * g, F * (g + 1))
        in_maps.append({
            "xT": xTs[b],
            "wqT": np.ascontiguousarray(Wq[fsl, :].T).astype(bf16),
            "wkT": np.ascontiguousarray(Wk[fsl, :].T).astype(bf16),
            "wvT": np.ascontiguousarray(Wv[fsl, :].T).astype(bf16),
            "woT": np.ascontiguousarray(Wo[:, fsl].T).astype(bf16),
            "tri": tri,
        })
    return in_maps


def combine_outputs(results):
    out = np.empty((B, S, E), np.float32)
    for b in range(B):
        acc = results[4 * b]["y"].astype(np.float32)
        for g in range(1, 4):
            acc = acc + results[4 * b + g]["y"].astype(np.float32)
        out[b] = acc
    return out


def kernel(x, Wq, Wk, Wv, Wo):
    import time as _time

    nc = _get_nc()
    in_maps = make_in_maps(x, Wq, Wk, Wv, Wo)
    last_exc = None
    for attempt in range(3):
        if attempt:
            # transient device wedge -- wait for recovery before retrying
            _time.sleep(30 * attempt)
        try:
            res = bass_utils.run_bass_kernel_spmd(
                nc, in_maps, core_ids=list(range(8))
            )
            return combine_outputs(res.results)
        except Exception as exc:
            last_exc = exc
    raise last_exc
